# revision 64
# baseline (speedup 1.0000x reference)
"""Trainium2 Bass kernel for nn_AttentionContextEncoder.

Data-parallel over 8 NeuronCores (batch 131072 -> 16384 rows/core).
Feature-major on chip: features on SBUF partitions, batch on the free dim.
Inputs are packed/transposed to feature-major bf16 on the host, the output
is written feature-major fp32 and transposed back on the host, so the
device does no layout transposes.

All matmuls run in bf16 (1 PE cycle/row) or float32r (1 cycle/row at
free>=256) instead of fp32 (4 cycles/row). Tokens are processed stacked
two-per-128-partition tile: groups (v,a), (s,t), (p). Cross-partition
reductions/broadcasts (softmax denominators, prob replication over
head_dim, LayerNorm stats) are PE matmuls against host-built selector
matrices. The LayerNorm mean subtraction is folded into the fusion-MLP
weights (correction term via an extra accumulated matmul), and gamma/beta
are folded into W1/b1.
"""

import sys

sys.path.insert(0, "/opt/trn_rl_repo")

import numpy as np
import ml_dtypes

BF16NP = ml_dtypes.bfloat16
FP8NP = ml_dtypes.float8_e4m3fn

import concourse.bass as bass
import concourse.mybir as mybir
import concourse.tile as tile
from concourse import bacc
from concourse.bass import ds

F32 = mybir.dt.float32
F32R = mybir.dt.float32r
FP8 = mybir.dt.float8e4
BF16 = mybir.dt.bfloat16
AF = mybir.ActivationFunctionType
ALU = mybir.AluOpType

B = 131072
NCORES = 8
R = B // NCORES          # rows per core = 16384
FD = 512                 # batch columns per pipeline tile
NT = R // FD             # tiles per core = 32
E = 64
H = 4
D = 16
S = 5                    # tokens: order (v, a, s, t, p)
EPS = 1e-3

# feature-major row ranges of the packed transposed input
FV, FA, FS, FT, FP = 14, 17, 7, 10, 51
OV, OA, OS, OT, OP = 0, 14, 31, 38, 64
F_ALL = OP + FP                           # 115 (rows 48:64 unused)

# token indices (our order) and reference row order (v,a,p,s,t) in W1
TOK = {"v": 0, "a": 1, "s": 2, "t": 3, "p": 4}
REFROW = [0, 1, 3, 4, 2]  # our token i -> reference token row

# 13 stacked score/AV mul-tiles: (qtile, ktile, vtile, top pair, bottom pair)
# pair = (sq, sk) in our token order; group g: 0 -> o_va, 1 -> o_st, 2 -> o_p
MULS = [
    ("Qva", "Kva", "Vva", (0, 0), (1, 1), 0),
    ("Qva", "Kav", "Vav", (0, 1), (1, 0), 0),
    ("Qva", "Kst", "Vst", (0, 2), (1, 3), 0),
    ("Qva", "Kts", "Vts", (0, 3), (1, 2), 0),
    ("Qva", "Kpp", "Vpp", (0, 4), (1, 4), 0),
    ("Qst", "Kva", "Vva", (2, 0), (3, 1), 1),
    ("Qst", "Kav", "Vav", (2, 1), (3, 0), 1),
    ("Qst", "Kst", "Vst", (2, 2), (3, 3), 1),
    ("Qst", "Kts", "Vts", (2, 3), (3, 2), 1),
    ("Qst", "Kpp", "Vpp", (2, 4), (3, 4), 1),
    ("Qpp", "Kva", "Vva", (4, 0), (4, 1), 2),
    ("Qpp", "Kst", "Vst", (4, 2), (4, 3), 2),
    ("Qpp", "Kpp", "Vpp", (4, 4), None, 2),
]
NM = len(MULS)
# QKV projection tiles, composed straight from the hidden layer (the token
# upsample U is folded into each lhsT): name -> (hidden source, const name)
QKV_TILES = [
    ("Qva", "hA", "WQva"), ("Qst", "hA", "WQst"), ("Qpp", "hP", "WQpp"),
    ("Kva", "hA", "WKva"), ("Kav", "hA", "WKav"), ("Kst", "hA", "WKst"),
    ("Kts", "hA", "WKts"), ("Kpp", "hP", "WKpp"),
    ("Vva", "hA", "WVva"), ("Vav", "hA", "WVav"), ("Vst", "hA", "WVst"),
    ("Vts", "hA", "WVts"), ("Vpp", "hP", "WVpp"),
]


def _build_constants(w):
    """Pack all weights/selectors into PE-friendly matrices (host, numpy).

    bf16 for bulk matmul weights, fp32 for biases / f32r-path selectors.
    """
    c = {}
    bf = lambda a: np.ascontiguousarray(a).astype(BF16NP)
    f32 = lambda a: np.ascontiguousarray(a, dtype=np.float32)

    Wq = np.asarray(w["Wq"], np.float32).reshape(E, H * D)
    Wk = np.asarray(w["Wk"], np.float32).reshape(E, H * D)
    Wv = np.asarray(w["Wvv"], np.float32).reshape(E, H * D)
    Wo = np.asarray(w["Wo"], np.float32).reshape(H * D, E)
    bq = np.asarray(w["bq"], np.float32).reshape(H * D)
    bk = np.asarray(w["bk"], np.float32).reshape(H * D)
    bv = np.asarray(w["bvv"], np.float32).reshape(H * D)

    # stage A: modality hidden layers. hA layout: s(0:16)|t(16:32)|v(32:64)|a(64:128)
    WA = np.zeros((48, 128), np.float32)
    WA[OV:OV + FV, 32:64] = w["Wv_p"]
    WA[OA:OA + FA, 64:128] = w["Wa_p"]
    WA[OS:OS + FS, 0:16] = w["Ws_p"]
    WA[OT:OT + FT, 16:32] = w["Wt_p"]
    c["WA"] = bf(WA)
    c["bA"] = f32(np.concatenate([w["bs_p"], w["bt_p"], w["bv_p"], w["ba_p"]])[:, None])
    WP = np.zeros((F_ALL, 32), np.float32)
    WP[OP:OP + FP, :] = w["Wp_p"]
    c["WP"] = bf(WP)
    c["bP"] = f32(np.asarray(w["bp_p"])[:, None])

    # token upsample matrices (stacked two per tile), folded into QKV below
    # and into the out-proj psum for the residual
    UVA = np.zeros((128, 128), np.float32)
    UVA[32:64, 0:64] = w["Wv_u"]
    UVA[64:128, 64:128] = w["Wa_u"]
    bUVA = np.concatenate([np.asarray(w["bv_u"]), np.asarray(w["ba_u"])]).astype(np.float32)
    UST = np.zeros((128, 128), np.float32)
    UST[0:16, 0:64] = w["Ws_u"]
    UST[16:32, 64:128] = w["Wt_u"]
    bUST = np.concatenate([np.asarray(w["bs_u"]), np.asarray(w["bt_u"])]).astype(np.float32)
    UP = np.asarray(w["Wp_u"], np.float32)
    bUP = np.asarray(w["bp_u"], np.float32)
    c["UVA"], c["UST"], c["UP"] = bf(UVA), bf(UST), bf(UP)

    # QKV lhsT variants: block-diag, anti-block (swapped halves), p-dup,
    # each precomposed with the relevant upsample: lhsT = U @ variant(W),
    # bias = variant(W).T @ bU + [b; b]
    def blockdiag(m):
        out = np.zeros((128, 128), np.float32)
        out[0:64, 0:64] = m
        out[64:128, 64:128] = m
        return out

    def antiblock(m):
        out = np.zeros((128, 128), np.float32)
        out[0:64, 64:128] = m
        out[64:128, 0:64] = m
        return out

    for nm, Wm, bm in (("Q", Wq, bq), ("K", Wk, bk), ("V", Wv, bv)):
        bd, ab = blockdiag(Wm), antiblock(Wm)
        dup = np.concatenate([Wm, Wm], axis=1)
        b2 = np.concatenate([bm, bm])
        c[f"W{nm}va"] = bf(UVA @ bd)
        c[f"W{nm}st"] = bf(UST @ bd)
        c[f"W{nm}pp"] = bf(UP @ dup)
        c[f"b{nm}va"] = f32((bd.T @ bUVA + b2)[:, None])
        c[f"b{nm}st"] = f32((bd.T @ bUST + b2)[:, None])
        c[f"b{nm}pp"] = f32((dup.T @ bUP + b2)[:, None])
        if nm in ("K", "V"):
            c[f"W{nm}av"] = bf(UVA @ ab)
            c[f"W{nm}ts"] = bf(UST @ ab)
            c[f"b{nm}av"] = f32((ab.T @ bUVA + b2)[:, None])
            c[f"b{nm}ts"] = f32((ab.T @ bUST + b2)[:, None])

    # score selectors: prod tile m -> score rows P(sq,sk)*4+h, scaled 1/sqrt(D)
    scl = 1.0 / np.sqrt(D)
    SELP = np.zeros((NM, 128, S * S * H), np.float32)
    REPA = np.zeros((NM, S * S * H, 128), np.float32)
    for m, (_, _, _, top, bot, _) in enumerate(MULS):
        for half, pair in ((0, top), (1, bot)):
            if pair is None:
                continue
            p = pair[0] * S + pair[1]
            for h in range(H):
                for d in range(D):
                    SELP[m, half * 64 + h * D + d, p * H + h] = scl
                    REPA[m, p * H + h, half * 64 + h * D + d] = 1.0
    c["SELP"] = bf(SELP.transpose(1, 0, 2).reshape(128, NM * S * S * H))
    c["REPA"] = bf(REPA.transpose(1, 0, 2).reshape(S * S * H, NM * 128))

    # replicated softmax denominator: out row (sq,sk,h) = sum_sk' exps[(sq,sk'),h]
    SELDR = np.zeros((S * S * H, S * S * H), np.float32)
    for sq in range(S):
        for h in range(H):
            for sk in range(S):
                for sk2 in range(S):
                    SELDR[(sq * S + sk2) * H + h, (sq * S + sk) * H + h] = 1.0
    c["SELDR"] = bf(SELDR)

    # out projection (stacked) + combined residual bias (bo + upsample bias).
    # WO1 is vertically stacked so the G2 fold (top half + bottom half)
    # happens inside the matmul. The upsample matmul accumulates into the
    # same psum, so h = psum + bRES in a single Act move.
    bo = np.asarray(w["bo"], np.float32)
    c["WO2"] = bf(blockdiag(Wo))
    c["WO1"] = bf(np.concatenate([Wo, Wo], axis=0))
    c["bRES0"] = f32((np.concatenate([bo, bo]) + bUVA)[:, None])
    c["bRES1"] = f32((np.concatenate([bo, bo]) + bUST)[:, None])
    c["bRES2"] = f32((bo + bUP)[:, None])

    # LayerNorm mean selectors: group tiles -> stat rows (token order v,a,s,t,p)
    for g, (rows, lo) in enumerate([(128, 0), (128, 2), (64, 4)]):
        sel = np.zeros((rows, S), np.float32)
        sel[0:64, lo] = 1.0 / E
        if g < 2:
            sel[64:128, lo + 1] = 1.0 / E
        c[f"SELM{g}"] = bf(sel)

    # replicate per-token inv-std over the 64 feature rows of each group
    RT0 = np.zeros((S, 128), np.float32)
    RT0[0, 0:64] = 1.0
    RT0[1, 64:128] = 1.0
    RT1 = np.zeros((S, 128), np.float32)
    RT1[2, 0:64] = 1.0
    RT1[3, 64:128] = 1.0
    RT2 = np.zeros((S, 64), np.float32)
    RT2[4, :] = 1.0
    c["RT0"], c["RT1"], c["RT2"] = bf(RT0), bf(RT1), bf(RT2)

    # fusion MLP with gamma/beta folded in; mean subtraction folded via CORR
    g = np.asarray(w["gamma"], np.float32)
    bt = np.asarray(w["beta"], np.float32)
    W1 = np.asarray(w["W1"], np.float32)
    W2 = np.asarray(w["W2"], np.float32)
    gt = np.tile(g, S)
    W1g = W1 * gt[:, None]
    b1p = np.asarray(w["b1"], np.float32) + np.tile(bt, S) @ W1
    # K-chunks in our token grouping: G0=(v,a) rows 0:128, G1=(s,t) rows
    # 192:320, G2=(p) rows 128:192 of reference order (v,a,p,s,t)
    c["W1A0"], c["W1B0"] = bf(W1g[0:128, 0:128]), bf(W1g[0:128, 128:256])
    c["W1A1"], c["W1B1"] = bf(W1g[192:320, 0:128]), bf(W1g[192:320, 128:256])
    c["W1A2"], c["W1B2"] = bf(W1g[128:192, 0:128]), bf(W1g[128:192, 128:256])
    # CORR[i, j] = -sum_e gamma[e] * W1[refrow(i)*64+e, j]; rhs is mi = mu*inv
    Cm = np.zeros((S, 256), np.float32)
    for i in range(S):
        r = REFROW[i]
        Cm[i] = g @ W1[r * E:(r + 1) * E]
    c["CORRa"], c["CORRb"] = bf(-Cm[:, 0:128]), bf(-Cm[:, 128:256])
    c["b1a"] = f32(b1p[0:128, None])
    c["b1b"] = f32(b1p[128:256, None])
    c["W2Aa"], c["W2Ba"] = bf(W2[0:128, 0:128]), bf(W2[128:256, 0:128])
    c["W2Ab"], c["W2Bb"] = bf(W2[0:128, 128:160]), bf(W2[128:256, 128:160])
    c["b2a"] = f32(np.asarray(w["b2"])[0:128, None])
    c["b2b"] = f32(np.asarray(w["b2"])[128:160, None])
    c["epsb"] = f32(np.full((S, 1), EPS))
    return c


_NPDT = {np.dtype(np.float32): F32, np.dtype(BF16NP): BF16, np.dtype(FP8NP): FP8}


def _patch_act_tables():
    """Make the act-table-load pass put Exp and Ln in the one table that has
    both (natural_log_exp_and_others). The pass greedily picks the first
    table containing a function; stripping Exp/Ln from the other tables
    (indices unchanged, so the emitted act_func_set_id stays valid) avoids
    two 1.3us table reloads per tile."""
    import concourse.bacc as bacc_mod
    import concourse.hw_specs as hw_specs
    if getattr(bacc_mod, "_act_tables_patched", False):
        return
    orig = hw_specs.get_activation_tables
    target = "natural_log_exp_and_others"

    def patched(arch):
        t = {k: set(v) for k, v in orig(arch).items()}
        if target in t:
            for k in t:
                if k != target:
                    t[k] = t[k] - {AF.Exp, AF.Ln}
        return t

    bacc_mod.get_activation_tables = patched
    bacc_mod._act_tables_patched = True


def _build_bass(const_specs):
    _patch_act_tables()
    nc = bacc.Bacc("TRN2", target_bir_lowering=False, debug=False,
                   num_devices=NCORES)
    din = {"xpack": nc.dram_tensor("xpack", (F_ALL, R), BF16, kind="ExternalInput")}
    for nm, (shp, dt) in const_specs.items():
        din[nm] = nc.dram_tensor(nm, shp, dt, kind="ExternalInput")
    dout = nc.dram_tensor("out", (160, R), F32, kind="ExternalOutput")

    # PSUM pools are split by pipeline phase so the cross-iteration ring
    # reuse pairs like-with-like (iteration i+1's front-phase matmuls wait
    # only on i+1-adjacent consumers, not on iteration i's tail). 8 banks:
    # front 2 + scores/den 1 + repa/ir 2 + late 2 + stats 1.
    with nc.allow_low_precision("bf16 throughout; tolerance 2e-2"), \
            tile.TileContext(nc) as tc, \
            tc.tile_pool(name="wp", bufs=1) as wp, \
            tc.tile_pool(name="xp", bufs=4) as xp, \
            tc.tile_pool(name="sp", bufs=3) as sp, \
            tc.tile_pool(name="pp", bufs=3) as pp, \
            tc.tile_pool(name="spo", bufs=2) as spo, \
            tc.tile_pool(name="psf", bufs=3, space="PSUM") as psfp, \
            tc.tile_pool(name="pssc", bufs=1, space="PSUM") as pssc, \
            tc.tile_pool(name="psr", bufs=2, space="PSUM") as psr, \
            tc.tile_pool(name="psl", bufs=2, space="PSUM") as psl:
        W = {}
        for nm, (shp, dt) in const_specs.items():
            t = wp.tile(list(shp), dt, tag=nm)
            nc.sync.dma_start(t[:], din[nm][:])
            W[nm] = t

        def emit_front(it):
            """DMA in -> stage A -> QKV -> score prods -> exp."""
            r0 = it * FD
            xin = xp.tile([F_ALL, FD], BF16, tag="xin")
            nc.sync.dma_start(xin[:], din["xpack"][:, r0:r0 + FD])

            ps_hA = psfp.tile([128, FD], F32, tag="f")
            nc.tensor.matmul(ps_hA[:], W["WA"][:], xin[0:48, :])
            hA = sp.tile([128, FD], BF16, tag="hA")
            nc.scalar.activation(hA[:], ps_hA[:], AF.Relu, bias=W["bA"][:])
            ps_hP = psfp.tile([32, FD], F32, tag="f")
            nc.tensor.matmul(ps_hP[:], W["WP"][OP:F_ALL, :], xin[OP:F_ALL, :])
            hP = sp.tile([32, FD], BF16, tag="hP")
            nc.scalar.activation(hP[:], ps_hP[:], AF.Relu, bias=W["bP"][:])

            hsrc = {"hA": hA, "hP": hP}
            qkv = {}
            for i, (tn, hn, wk) in enumerate(QKV_TILES):
                ps_t = psfp.tile([128, FD], F32, tag="f")
                nc.tensor.matmul(ps_t[:], W[wk][:], hsrc[hn][:])
                t = sp.tile([128, FD], BF16, tag=tn)
                bias = W["b" + tn]
                if i in (4, 9):
                    nc.vector.tensor_scalar_add(t[:], ps_t[:], bias[:])
                else:
                    nc.scalar.activation(t[:], ps_t[:], AF.Identity, bias=bias[:])
                qkv[tn] = t

            ps_sc = pssc.tile([S * S * H, FD], F32, tag="scd")
            for m, (qn, kn, _, _, _, _) in enumerate(MULS):
                prod = pp.tile([128, FD], BF16, tag="prod", bufs=8)
                if m in (1, 3, 5, 7, 9, 11, 12):
                    nc.vector.tensor_mul(prod[:], qkv[qn][:], qkv[kn][:])
                else:
                    nc.gpsimd.tensor_mul(prod[:], qkv[qn][:], qkv[kn][:])
                nc.tensor.matmul(ps_sc[:], W["SELP"][:, ds(m * S * S * H, S * S * H)],
                                 prod[:], start=(m == 0), stop=(m == NM - 1))
            exps = sp.tile([S * S * H, FD], BF16, tag="exps")
            nc.scalar.activation(exps[:], ps_sc[:], AF.Exp)
            return {"it": it, "hA": hA, "hP": hP, "qkv": qkv, "exps": exps,
                    "hsrc": hsrc}

        def emit_back(fs):
            """softmax normalize -> AV -> out-proj/residual -> LN -> MLP -> out."""
            it, exps, qkv, hsrc = fs["it"], fs["exps"], fs["qkv"], fs["hsrc"]
            r0 = it * FD
            ps_dr = pssc.tile([S * S * H, FD], F32, tag="scd")
            nc.tensor.matmul(ps_dr[:], W["SELDR"][:], exps[:])
            rden = sp.tile([S * S * H, FD], BF16, tag="rden")
            nc.vector.reciprocal(rden[:], ps_dr[:])
            pnorm = sp.tile([S * S * H, FD], BF16, tag="pnorm")
            nc.vector.tensor_mul(pnorm[:], exps[:], rden[:])

            # ---- o = sum_sk probs * v  (probs replicated over d via PE;
            # prod2 = psum * v on DVE). The per-group sum over sk happens
            # inside the out-proj matmul accumulation (WO is linear), which
            # also accumulates the upsample x for the residual.
            prods = []
            for m, (_, _, vn, _, _, _) in enumerate(MULS):
                ps_ar = psr.tile([128, FD], F32, tag="r")
                nc.tensor.matmul(ps_ar[:], W["REPA"][:, ds(m * 128, 128)], pnorm[:])
                pr = pp.tile([128, FD], BF16, tag="prod2", bufs=10)
                nc.vector.tensor_mul(pr[:], ps_ar[:], qkv[vn][:])
                prods.append(pr)

            hs = []
            for g, (won, un, hn) in enumerate([("WO2", "UVA", "hA"),
                                               ("WO2", "UST", "hA"),
                                               ("WO1", "UP", "hP")]):
                rows = 128 if g < 2 else 64
                gp = [p for m, p in enumerate(prods) if MULS[m][5] == g]
                ps_at = psl.tile([rows, FD], F32, tag="l")
                for j, pr in enumerate(gp):
                    nc.tensor.matmul(ps_at[:], W[won][:], pr[:],
                                     start=(j == 0), stop=False)
                nc.tensor.matmul(ps_at[:], W[un][:], hsrc[hn][:],
                                 start=False, stop=True)
                ht = sp.tile([rows, FD], BF16, tag=f"h{g}")
                if g == 1:
                    nc.vector.tensor_scalar_add(ht[:], ps_at[:], W[f"bRES{g}"][:])
                else:
                    nc.scalar.activation(ht[:], ps_at[:], AF.Identity,
                                         bias=W[f"bRES{g}"][:])
                hs.append(ht)

            # ---- LayerNorm stats (mean folded into MLP; inv-std applied
            # here). mu/ms psum tiles share the scores/den ring (adjacent
            # stage reuse only).
            ps_mu = psl.tile([S, FD], F32, tag="l")
            ps_ms = psl.tile([S, FD], F32, tag="l")
            sqs = []
            for g in range(3):
                sq = pp.tile([128 if g < 2 else 64, FD], BF16, tag=f"sq{g}")
                nc.scalar.square(sq[:], hs[g][:])
                sqs.append(sq)
            for g in range(3):
                nc.tensor.matmul(ps_mu[:], W[f"SELM{g}"][:], hs[g][:],
                                 start=(g == 0), stop=(g == 2))
            for g in range(3):
                nc.tensor.matmul(ps_ms[:], W[f"SELM{g}"][:], sqs[g][:],
                                 start=(g == 0), stop=(g == 2))
            mu_sb = sp.tile([S, FD], BF16, tag="mu")
            nc.scalar.activation(mu_sb[:], ps_mu[:], AF.Identity)
            mu2 = pp.tile([S, FD], BF16, tag="mu2")
            nc.scalar.square(mu2[:], ps_mu[:])
            var = pp.tile([S, FD], BF16, tag="var")
            nc.vector.tensor_sub(var[:], ps_ms[:], mu2[:])
            # inv-std = exp(-0.5 * ln(var + eps)): keeps every Act function in
            # the natural_log_exp table (no act-table reloads)
            lnv = pp.tile([S, FD], F32, tag="lnv")
            nc.scalar.activation(lnv[:], var[:], AF.Ln, bias=W["epsb"][:])
            inv = sp.tile([S, FD], BF16, tag="inv")
            nc.scalar.activation(inv[:], lnv[:], AF.Exp, scale=-0.5)
            mi = sp.tile([S, FD], BF16, tag="mi")
            nc.gpsimd.tensor_mul(mi[:], mu_sb[:], inv[:])

            # h' = h * inv_replicated (selector matmul for replication)
            hps = []
            for g, rt in enumerate(["RT0", "RT1", "RT2"]):
                rows = 128 if g < 2 else 64
                ps_ir = psr.tile([rows, FD], F32, tag="r")
                nc.tensor.matmul(ps_ir[:], W[rt][:], inv[:])
                hp = sp.tile([rows, FD], BF16, tag=f"hp{g}")
                nc.vector.tensor_mul(hp[:], ps_ir[:], hs[g][:])
                hps.append(hp)

            # ---- fusion MLP (gamma/beta folded; -mu correction via CORR @ mi)
            ps_f1a = psl.tile([128, FD], F32, tag="l")
            ps_f1b = psl.tile([128, FD], F32, tag="l")
            for psf, wl, corr in ((ps_f1a, ["W1A0", "W1A1", "W1A2"], "CORRa"),
                                  (ps_f1b, ["W1B0", "W1B1", "W1B2"], "CORRb")):
                for g in range(3):
                    nc.tensor.matmul(psf[:], W[wl[g]][:], hps[g][:],
                                     start=(g == 0), stop=False)
                nc.tensor.matmul(psf[:], W[corr][:], mi[:],
                                 start=False, stop=True)
            f1a = sp.tile([128, FD], BF16, tag="f1a")
            nc.scalar.activation(f1a[:], ps_f1a[:], AF.Relu, bias=W["b1a"][:])
            f1b = sp.tile([128, FD], BF16, tag="f1b")
            nc.scalar.activation(f1b[:], ps_f1b[:], AF.Relu, bias=W["b1b"][:])

            ps_o1 = psl.tile([128, FD], F32, tag="l")
            nc.tensor.matmul(ps_o1[:], W["W2Aa"][:], f1a[:], start=True, stop=False)
            nc.tensor.matmul(ps_o1[:], W["W2Ba"][:], f1b[:], start=False, stop=True)
            ps_o2 = psl.tile([32, FD], F32, tag="l")
            nc.tensor.matmul(ps_o2[:], W["W2Ab"][:], f1a[:], start=True, stop=False)
            nc.tensor.matmul(ps_o2[:], W["W2Bb"][:], f1b[:], start=False, stop=True)
            oo1 = spo.tile([128, FD], F32, tag="oo1")
            nc.scalar.activation(oo1[:], ps_o1[:], AF.Relu, bias=W["b2a"][:])
            oo2 = spo.tile([32, FD], F32, tag="oo2")
            nc.scalar.activation(oo2[:], ps_o2[:], AF.Relu, bias=W["b2b"][:])

            nc.sync.dma_start(dout[0:128, r0:r0 + FD], oo1[:])
            nc.sync.dma_start(dout[128:160, r0:r0 + FD], oo2[:])

        # software pipeline: emit back(i-1) before front(i) so every
        # engine's in-order queue alternates ready back-work with front
        # work instead of draining a whole iteration at a time
        prev = None
        for it in range(NT + 1):
            if prev is not None:
                emit_back(prev)
            prev = emit_front(it) if it < NT else None

    nc.compile()
    return nc


_CACHE = {}


def _pack_inputs(w):
    """Feature-major bf16 [F_ALL, B] packed input (host-side transpose)."""
    xp = np.zeros((F_ALL, B), dtype=BF16NP)
    for nm, off, f in [("visual", OV, FV), ("audio", OA, FA), ("spatial", OS, FS),
                       ("time", OT, FT), ("pose", OP, FP)]:
        xp[off:off + f, :] = np.asarray(w[nm], np.float32).T.astype(BF16NP)
    return xp


def kernel(**inputs):
    w = {k: np.asarray(v) for k, v in inputs.items()}
    consts = _build_constants(w)
    xpack = _pack_inputs(w)

    const_specs = {k: (v.shape, _NPDT[v.dtype]) for k, v in consts.items()}
    key = tuple(sorted((k, s, str(d)) for k, (s, d) in const_specs.items()))
    if key not in _CACHE:
        _CACHE[key] = _build_bass(const_specs)
    nc = _CACHE[key]

    from concourse.bass_utils import run_bass_kernel_spmd

    in_maps = []
    for c in range(NCORES):
        m = {"xpack": np.ascontiguousarray(xpack[:, c * R:(c + 1) * R])}
        m.update(consts)
        in_maps.append(m)

    res = run_bass_kernel_spmd(nc, in_maps, core_ids=list(range(NCORES)))
    out = np.concatenate([r["out"] for r in res.results], axis=1)  # [160, B]
    return np.ascontiguousarray(out.T, dtype=np.float32)


# revision 66
# speedup vs baseline: 1.0550x; 1.0550x over previous
"""Trainium2 Bass kernel for nn_AttentionContextEncoder.

Data-parallel over 8 NeuronCores (batch 131072 -> 16384 rows/core).
Feature-major on chip: features on SBUF partitions, batch on the free dim.
Inputs are packed/transposed to feature-major bf16 on the host, the output
is written feature-major fp32 and transposed back on the host, so the
device does no layout transposes.

All matmuls run in bf16 (1 PE cycle/row) or float32r (1 cycle/row at
free>=256) instead of fp32 (4 cycles/row). Tokens are processed stacked
two-per-128-partition tile: groups (v,a), (s,t), (p). Cross-partition
reductions/broadcasts (softmax denominators, prob replication over
head_dim, LayerNorm stats) are PE matmuls against host-built selector
matrices. The LayerNorm mean subtraction is folded into the fusion-MLP
weights (correction term via an extra accumulated matmul), and gamma/beta
are folded into W1/b1.
"""

import sys

sys.path.insert(0, "/opt/trn_rl_repo")

import numpy as np
import ml_dtypes

BF16NP = ml_dtypes.bfloat16
FP8NP = ml_dtypes.float8_e4m3fn

import concourse.bass as bass
import concourse.mybir as mybir
import concourse.tile as tile
from concourse import bacc
from concourse.bass import ds

F32 = mybir.dt.float32
F32R = mybir.dt.float32r
FP8 = mybir.dt.float8e4
BF16 = mybir.dt.bfloat16
AF = mybir.ActivationFunctionType
ALU = mybir.AluOpType

B = 131072
NCORES = 8
R = B // NCORES          # rows per core = 16384
FD = 512                 # batch columns per pipeline tile
NT = R // FD             # tiles per core = 32
E = 64
H = 4
D = 16
S = 5                    # tokens: order (v, a, s, t, p)
EPS = 1e-3

# feature-major row ranges of the packed transposed input
FV, FA, FS, FT, FP = 14, 17, 7, 10, 51
OV, OA, OS, OT, OP = 0, 14, 31, 38, 64
F_ALL = OP + FP                           # 115 (rows 48:64 unused)

# token indices (our order) and reference row order (v,a,p,s,t) in W1
TOK = {"v": 0, "a": 1, "s": 2, "t": 3, "p": 4}
REFROW = [0, 1, 3, 4, 2]  # our token i -> reference token row

# 13 stacked score/AV mul-tiles: (qtile, ktile, vtile, top pair, bottom pair)
# pair = (sq, sk) in our token order; group g: 0 -> o_va, 1 -> o_st, 2 -> o_p
MULS = [
    ("Qva", "Kva", "Vva", (0, 0), (1, 1), 0),
    ("Qva", "Kav", "Vav", (0, 1), (1, 0), 0),
    ("Qva", "Kst", "Vst", (0, 2), (1, 3), 0),
    ("Qva", "Kts", "Vts", (0, 3), (1, 2), 0),
    ("Qva", "Kpp", "Vpp", (0, 4), (1, 4), 0),
    ("Qst", "Kva", "Vva", (2, 0), (3, 1), 1),
    ("Qst", "Kav", "Vav", (2, 1), (3, 0), 1),
    ("Qst", "Kst", "Vst", (2, 2), (3, 3), 1),
    ("Qst", "Kts", "Vts", (2, 3), (3, 2), 1),
    ("Qst", "Kpp", "Vpp", (2, 4), (3, 4), 1),
    ("Qpp", "Kva", "Vva", (4, 0), (4, 1), 2),
    ("Qpp", "Kst", "Vst", (4, 2), (4, 3), 2),
    ("Qpp", "Kpp", "Vpp", (4, 4), None, 2),
]
NM = len(MULS)
# QKV projection tiles, composed straight from the hidden layer (the token
# upsample U is folded into each lhsT): name -> (hidden source, const name)
QKV_TILES = [
    ("Qva", "hA", "WQva"), ("Qst", "hA", "WQst"), ("Qpp", "hP", "WQpp"),
    ("Kva", "hA", "WKva"), ("Kav", "hA", "WKav"), ("Kst", "hA", "WKst"),
    ("Kts", "hA", "WKts"), ("Kpp", "hP", "WKpp"),
    ("Vva", "hA", "WVva"), ("Vav", "hA", "WVav"), ("Vst", "hA", "WVst"),
    ("Vts", "hA", "WVts"), ("Vpp", "hP", "WVpp"),
]


def _build_constants(w):
    """Pack all weights/selectors into PE-friendly matrices (host, numpy).

    bf16 for bulk matmul weights, fp32 for biases / f32r-path selectors.
    """
    c = {}
    bf = lambda a: np.ascontiguousarray(a).astype(BF16NP)
    f32 = lambda a: np.ascontiguousarray(a, dtype=np.float32)

    Wq = np.asarray(w["Wq"], np.float32).reshape(E, H * D)
    Wk = np.asarray(w["Wk"], np.float32).reshape(E, H * D)
    Wv = np.asarray(w["Wvv"], np.float32).reshape(E, H * D)
    Wo = np.asarray(w["Wo"], np.float32).reshape(H * D, E)
    bq = np.asarray(w["bq"], np.float32).reshape(H * D)
    bk = np.asarray(w["bk"], np.float32).reshape(H * D)
    bv = np.asarray(w["bvv"], np.float32).reshape(H * D)

    # stage A: modality hidden layers. hA layout: s(0:16)|t(16:32)|v(32:64)|a(64:128)
    WA = np.zeros((48, 128), np.float32)
    WA[OV:OV + FV, 32:64] = w["Wv_p"]
    WA[OA:OA + FA, 64:128] = w["Wa_p"]
    WA[OS:OS + FS, 0:16] = w["Ws_p"]
    WA[OT:OT + FT, 16:32] = w["Wt_p"]
    c["WA"] = bf(WA)
    c["bA"] = f32(np.concatenate([w["bs_p"], w["bt_p"], w["bv_p"], w["ba_p"]])[:, None])
    WP = np.zeros((F_ALL, 32), np.float32)
    WP[OP:OP + FP, :] = w["Wp_p"]
    c["WP"] = bf(WP)
    c["bP"] = f32(np.asarray(w["bp_p"])[:, None])

    # token upsample matrices (stacked two per tile), folded into QKV below
    # and into the out-proj psum for the residual
    UVA = np.zeros((128, 128), np.float32)
    UVA[32:64, 0:64] = w["Wv_u"]
    UVA[64:128, 64:128] = w["Wa_u"]
    bUVA = np.concatenate([np.asarray(w["bv_u"]), np.asarray(w["ba_u"])]).astype(np.float32)
    UST = np.zeros((128, 128), np.float32)
    UST[0:16, 0:64] = w["Ws_u"]
    UST[16:32, 64:128] = w["Wt_u"]
    bUST = np.concatenate([np.asarray(w["bs_u"]), np.asarray(w["bt_u"])]).astype(np.float32)
    UP = np.asarray(w["Wp_u"], np.float32)
    bUP = np.asarray(w["bp_u"], np.float32)
    c["UVA"], c["UST"], c["UP"] = bf(UVA), bf(UST), bf(UP)

    # QKV lhsT variants: block-diag, anti-block (swapped halves), p-dup,
    # each precomposed with the relevant upsample: lhsT = U @ variant(W),
    # bias = variant(W).T @ bU + [b; b]
    def blockdiag(m):
        out = np.zeros((128, 128), np.float32)
        out[0:64, 0:64] = m
        out[64:128, 64:128] = m
        return out

    def antiblock(m):
        out = np.zeros((128, 128), np.float32)
        out[0:64, 64:128] = m
        out[64:128, 0:64] = m
        return out

    for nm, Wm, bm in (("Q", Wq, bq), ("K", Wk, bk), ("V", Wv, bv)):
        bd, ab = blockdiag(Wm), antiblock(Wm)
        dup = np.concatenate([Wm, Wm], axis=1)
        b2 = np.concatenate([bm, bm])
        c[f"W{nm}va"] = bf(UVA @ bd)
        c[f"W{nm}st"] = bf(UST @ bd)
        c[f"W{nm}pp"] = bf(UP @ dup)
        c[f"b{nm}va"] = f32((bd.T @ bUVA + b2)[:, None])
        c[f"b{nm}st"] = f32((bd.T @ bUST + b2)[:, None])
        c[f"b{nm}pp"] = f32((dup.T @ bUP + b2)[:, None])
        if nm in ("K", "V"):
            c[f"W{nm}av"] = bf(UVA @ ab)
            c[f"W{nm}ts"] = bf(UST @ ab)
            c[f"b{nm}av"] = f32((ab.T @ bUVA + b2)[:, None])
            c[f"b{nm}ts"] = f32((ab.T @ bUST + b2)[:, None])

    # score selectors: prod tile m -> score rows P(sq,sk)*4+h, scaled 1/sqrt(D)
    scl = 1.0 / np.sqrt(D)
    SELP = np.zeros((NM, 128, S * S * H), np.float32)
    REPA = np.zeros((NM, S * S * H, 128), np.float32)
    for m, (_, _, _, top, bot, _) in enumerate(MULS):
        for half, pair in ((0, top), (1, bot)):
            if pair is None:
                continue
            p = pair[0] * S + pair[1]
            for h in range(H):
                for d in range(D):
                    SELP[m, half * 64 + h * D + d, p * H + h] = scl
                    REPA[m, p * H + h, half * 64 + h * D + d] = 1.0
    c["SELP"] = bf(SELP.transpose(1, 0, 2).reshape(128, NM * S * S * H))
    c["REPA"] = bf(REPA.transpose(1, 0, 2).reshape(S * S * H, NM * 128))

    # replicated softmax denominator: out row (sq,sk,h) = sum_sk' exps[(sq,sk'),h]
    SELDR = np.zeros((S * S * H, S * S * H), np.float32)
    for sq in range(S):
        for h in range(H):
            for sk in range(S):
                for sk2 in range(S):
                    SELDR[(sq * S + sk2) * H + h, (sq * S + sk) * H + h] = 1.0
    c["SELDR"] = bf(SELDR)

    # out projection (stacked) + combined residual bias (bo + upsample bias).
    # WO1 is vertically stacked so the G2 fold (top half + bottom half)
    # happens inside the matmul. The upsample matmul accumulates into the
    # same psum, so h = psum + bRES in a single Act move.
    bo = np.asarray(w["bo"], np.float32)
    c["WO2"] = bf(blockdiag(Wo))
    c["WO1"] = bf(np.concatenate([Wo, Wo], axis=0))
    c["bRES0"] = f32((np.concatenate([bo, bo]) + bUVA)[:, None])
    c["bRES1"] = f32((np.concatenate([bo, bo]) + bUST)[:, None])
    c["bRES2"] = f32((bo + bUP)[:, None])

    # LayerNorm mean selectors: group tiles -> stat rows (token order v,a,s,t,p)
    for g, (rows, lo) in enumerate([(128, 0), (128, 2), (64, 4)]):
        sel = np.zeros((rows, S), np.float32)
        sel[0:64, lo] = 1.0 / E
        if g < 2:
            sel[64:128, lo + 1] = 1.0 / E
        c[f"SELM{g}"] = bf(sel)

    # replicate per-token inv-std over the 64 feature rows of each group
    RT0 = np.zeros((S, 128), np.float32)
    RT0[0, 0:64] = 1.0
    RT0[1, 64:128] = 1.0
    RT1 = np.zeros((S, 128), np.float32)
    RT1[2, 0:64] = 1.0
    RT1[3, 64:128] = 1.0
    RT2 = np.zeros((S, 64), np.float32)
    RT2[4, :] = 1.0
    c["RT0"], c["RT1"], c["RT2"] = bf(RT0), bf(RT1), bf(RT2)

    # fusion MLP with gamma/beta folded in; mean subtraction folded via CORR
    g = np.asarray(w["gamma"], np.float32)
    bt = np.asarray(w["beta"], np.float32)
    W1 = np.asarray(w["W1"], np.float32)
    W2 = np.asarray(w["W2"], np.float32)
    gt = np.tile(g, S)
    W1g = W1 * gt[:, None]
    b1p = np.asarray(w["b1"], np.float32) + np.tile(bt, S) @ W1
    # K-chunks in our token grouping: G0=(v,a) rows 0:128, G1=(s,t) rows
    # 192:320, G2=(p) rows 128:192 of reference order (v,a,p,s,t)
    c["W1A0"], c["W1B0"] = bf(W1g[0:128, 0:128]), bf(W1g[0:128, 128:256])
    c["W1A1"], c["W1B1"] = bf(W1g[192:320, 0:128]), bf(W1g[192:320, 128:256])
    c["W1A2"], c["W1B2"] = bf(W1g[128:192, 0:128]), bf(W1g[128:192, 128:256])
    # CORR[i, j] = -sum_e gamma[e] * W1[refrow(i)*64+e, j]; rhs is mi = mu*inv
    Cm = np.zeros((S, 256), np.float32)
    for i in range(S):
        r = REFROW[i]
        Cm[i] = g @ W1[r * E:(r + 1) * E]
    c["CORRa"], c["CORRb"] = bf(-Cm[:, 0:128]), bf(-Cm[:, 128:256])
    c["b1a"] = f32(b1p[0:128, None])
    c["b1b"] = f32(b1p[128:256, None])
    c["W2Aa"], c["W2Ba"] = bf(W2[0:128, 0:128]), bf(W2[128:256, 0:128])
    c["W2Ab"], c["W2Bb"] = bf(W2[0:128, 128:160]), bf(W2[128:256, 128:160])
    c["b2a"] = f32(np.asarray(w["b2"])[0:128, None])
    c["b2b"] = f32(np.asarray(w["b2"])[128:160, None])
    c["epsb"] = f32(np.full((S, 1), EPS))
    return c


_NPDT = {np.dtype(np.float32): F32, np.dtype(BF16NP): BF16, np.dtype(FP8NP): FP8}


def _patch_act_tables():
    """Make the act-table-load pass put Exp and Ln in the one table that has
    both (natural_log_exp_and_others). The pass greedily picks the first
    table containing a function; stripping Exp/Ln from the other tables
    (indices unchanged, so the emitted act_func_set_id stays valid) avoids
    two 1.3us table reloads per tile."""
    import concourse.bacc as bacc_mod
    import concourse.hw_specs as hw_specs
    if getattr(bacc_mod, "_act_tables_patched", False):
        return
    orig = hw_specs.get_activation_tables
    target = "natural_log_exp_and_others"

    def patched(arch):
        t = {k: set(v) for k, v in orig(arch).items()}
        if target in t:
            for k in t:
                if k != target:
                    t[k] = t[k] - {AF.Exp, AF.Ln}
        return t

    bacc_mod.get_activation_tables = patched
    bacc_mod._act_tables_patched = True


def _pack_blob(consts):
    """Lay all constants into one [128, BLOB_BYTES] uint8 blob (4B aligned).
    Returns (blob, specs{name: (shape, mybir_dt, byte_off, nbytes_per_row)})."""
    specs = {}
    off = 0
    for nm, arr in consts.items():
        nbytes = arr.shape[-1] * arr.dtype.itemsize if arr.ndim == 2 else arr.dtype.itemsize
        rows = arr.shape[0]
        specs[nm] = (arr.shape, _NPDT[arr.dtype], off, rows, nbytes)
        off += (nbytes + 3) & ~3
    blob = np.zeros((128, off), np.uint8)
    for nm, arr in consts.items():
        shp, dt, o, rows, nbytes = specs[nm]
        blob[0:rows, o:o + nbytes] = np.ascontiguousarray(arr).view(np.uint8).reshape(rows, nbytes)
    return blob, {nm: (shp, dt, o, nb) for nm, (shp, dt, o, rows, nb) in specs.items()}


def _build_bass(const_specs, blob_bytes):
    global BLOB_BYTES
    BLOB_BYTES = blob_bytes
    _patch_act_tables()
    nc = bacc.Bacc("TRN2", target_bir_lowering=False, debug=False,
                   num_devices=NCORES)
    din = {"xpack": nc.dram_tensor("xpack", (F_ALL, R), BF16, kind="ExternalInput"),
           "wblob": nc.dram_tensor("wblob", (128, BLOB_BYTES), mybir.dt.uint8,
                                   kind="ExternalInput")}
    dout = nc.dram_tensor("out", (160, R), F32, kind="ExternalOutput")

    # PSUM pools are split by pipeline phase so the cross-iteration ring
    # reuse pairs like-with-like (iteration i+1's front-phase matmuls wait
    # only on i+1-adjacent consumers, not on iteration i's tail). 8 banks:
    # front 2 + scores/den 1 + repa/ir 2 + late 2 + stats 1.
    with nc.allow_low_precision("bf16 throughout; tolerance 2e-2"), \
            tile.TileContext(nc) as tc, \
            tc.tile_pool(name="wp", bufs=1) as wp, \
            tc.tile_pool(name="xp", bufs=4) as xp, \
            tc.tile_pool(name="sp", bufs=3) as sp, \
            tc.tile_pool(name="pp", bufs=3) as pp, \
            tc.tile_pool(name="spo", bufs=2) as spo, \
            tc.tile_pool(name="psf", bufs=3, space="PSUM") as psfp, \
            tc.tile_pool(name="pssc", bufs=1, space="PSUM") as pssc, \
            tc.tile_pool(name="psr", bufs=2, space="PSUM") as psr, \
            tc.tile_pool(name="psl", bufs=2, space="PSUM") as psl:
        # all constants arrive in ONE blob DMA (per-const DMAs serialized
        # ~625ns each on HWDGE and idled the whole pipeline for ~45us at
        # startup); each weight is a bitcast view into the blob tile
        blob = wp.tile([128, BLOB_BYTES], mybir.dt.uint8, tag="wblob")
        nc.sync.dma_start(blob[:], din["wblob"][:])
        W = {}
        for nm, (shp, dt, off, nbytes) in const_specs.items():
            W[nm] = blob[0:shp[0], off:off + nbytes].bitcast(dt)

        def emit_front(it):
            """DMA in -> stage A -> QKV -> score prods -> exp."""
            r0 = it * FD
            xin = xp.tile([F_ALL, FD], BF16, tag="xin")
            nc.sync.dma_start(xin[:], din["xpack"][:, r0:r0 + FD])

            ps_hA = psfp.tile([128, FD], F32, tag="f")
            nc.tensor.matmul(ps_hA[:], W["WA"][:], xin[0:48, :])
            hA = sp.tile([128, FD], BF16, tag="hA")
            nc.scalar.activation(hA[:], ps_hA[:], AF.Relu, bias=W["bA"][:])
            ps_hP = psfp.tile([32, FD], F32, tag="f")
            nc.tensor.matmul(ps_hP[:], W["WP"][OP:F_ALL, :], xin[OP:F_ALL, :])
            hP = sp.tile([32, FD], BF16, tag="hP")
            nc.scalar.activation(hP[:], ps_hP[:], AF.Relu, bias=W["bP"][:])

            hsrc = {"hA": hA, "hP": hP}
            qkv = {}
            for i, (tn, hn, wk) in enumerate(QKV_TILES):
                ps_t = psfp.tile([128, FD], F32, tag="f")
                nc.tensor.matmul(ps_t[:], W[wk][:], hsrc[hn][:])
                t = sp.tile([128, FD], BF16, tag=tn)
                bias = W["b" + tn]
                if i in (4, 9):
                    nc.vector.tensor_scalar_add(t[:], ps_t[:], bias[:])
                else:
                    nc.scalar.activation(t[:], ps_t[:], AF.Identity, bias=bias[:])
                qkv[tn] = t

            ps_sc = pssc.tile([S * S * H, FD], F32, tag="scd")
            for m, (qn, kn, _, _, _, _) in enumerate(MULS):
                prod = pp.tile([128, FD], BF16, tag="prod", bufs=8)
                if m in (1, 3, 5, 7, 9, 11, 12):
                    nc.vector.tensor_mul(prod[:], qkv[qn][:], qkv[kn][:])
                else:
                    nc.gpsimd.tensor_mul(prod[:], qkv[qn][:], qkv[kn][:])
                nc.tensor.matmul(ps_sc[:], W["SELP"][:, ds(m * S * S * H, S * S * H)],
                                 prod[:], start=(m == 0), stop=(m == NM - 1))
            exps = sp.tile([S * S * H, FD], BF16, tag="exps")
            nc.scalar.activation(exps[:], ps_sc[:], AF.Exp)
            return {"it": it, "hA": hA, "hP": hP, "qkv": qkv, "exps": exps,
                    "hsrc": hsrc}

        def emit_back(fs):
            """softmax normalize -> AV -> out-proj/residual -> LN -> MLP -> out."""
            it, exps, qkv, hsrc = fs["it"], fs["exps"], fs["qkv"], fs["hsrc"]
            r0 = it * FD
            ps_dr = pssc.tile([S * S * H, FD], F32, tag="scd")
            nc.tensor.matmul(ps_dr[:], W["SELDR"][:], exps[:])
            rden = sp.tile([S * S * H, FD], BF16, tag="rden")
            nc.vector.reciprocal(rden[:], ps_dr[:])
            pnorm = sp.tile([S * S * H, FD], BF16, tag="pnorm")
            nc.vector.tensor_mul(pnorm[:], exps[:], rden[:])

            # ---- o = sum_sk probs * v  (probs replicated over d via PE;
            # prod2 = psum * v on DVE). The per-group sum over sk happens
            # inside the out-proj matmul accumulation (WO is linear), which
            # also accumulates the upsample x for the residual.
            prods = []
            for m, (_, _, vn, _, _, _) in enumerate(MULS):
                ps_ar = psr.tile([128, FD], F32, tag="r")
                nc.tensor.matmul(ps_ar[:], W["REPA"][:, ds(m * 128, 128)], pnorm[:])
                pr = pp.tile([128, FD], BF16, tag="prod2", bufs=10)
                nc.vector.tensor_mul(pr[:], ps_ar[:], qkv[vn][:])
                prods.append(pr)

            hs = []
            for g, (won, un, hn) in enumerate([("WO2", "UVA", "hA"),
                                               ("WO2", "UST", "hA"),
                                               ("WO1", "UP", "hP")]):
                rows = 128 if g < 2 else 64
                gp = [p for m, p in enumerate(prods) if MULS[m][5] == g]
                ps_at = psl.tile([rows, FD], F32, tag="l")
                for j, pr in enumerate(gp):
                    nc.tensor.matmul(ps_at[:], W[won][:], pr[:],
                                     start=(j == 0), stop=False)
                nc.tensor.matmul(ps_at[:], W[un][:], hsrc[hn][:],
                                 start=False, stop=True)
                ht = sp.tile([rows, FD], BF16, tag=f"h{g}")
                if g == 1:
                    nc.vector.tensor_scalar_add(ht[:], ps_at[:], W[f"bRES{g}"][:])
                else:
                    nc.scalar.activation(ht[:], ps_at[:], AF.Identity,
                                         bias=W[f"bRES{g}"][:])
                hs.append(ht)

            # ---- LayerNorm stats (mean folded into MLP; inv-std applied
            # here). mu/ms psum tiles share the scores/den ring (adjacent
            # stage reuse only).
            ps_mu = psl.tile([S, FD], F32, tag="l")
            ps_ms = psl.tile([S, FD], F32, tag="l")
            sqs = []
            for g in range(3):
                sq = pp.tile([128 if g < 2 else 64, FD], BF16, tag=f"sq{g}")
                nc.scalar.square(sq[:], hs[g][:])
                sqs.append(sq)
            for g in range(3):
                nc.tensor.matmul(ps_mu[:], W[f"SELM{g}"][:], hs[g][:],
                                 start=(g == 0), stop=(g == 2))
            for g in range(3):
                nc.tensor.matmul(ps_ms[:], W[f"SELM{g}"][:], sqs[g][:],
                                 start=(g == 0), stop=(g == 2))
            mu_sb = sp.tile([S, FD], BF16, tag="mu")
            nc.scalar.activation(mu_sb[:], ps_mu[:], AF.Identity)
            mu2 = pp.tile([S, FD], BF16, tag="mu2")
            nc.scalar.square(mu2[:], ps_mu[:])
            var = pp.tile([S, FD], BF16, tag="var")
            nc.vector.tensor_sub(var[:], ps_ms[:], mu2[:])
            # inv-std = exp(-0.5 * ln(var + eps)): keeps every Act function in
            # the natural_log_exp table (no act-table reloads)
            lnv = pp.tile([S, FD], F32, tag="lnv")
            nc.scalar.activation(lnv[:], var[:], AF.Ln, bias=W["epsb"][:])
            inv = sp.tile([S, FD], BF16, tag="inv")
            nc.scalar.activation(inv[:], lnv[:], AF.Exp, scale=-0.5)
            mi = sp.tile([S, FD], BF16, tag="mi")
            nc.gpsimd.tensor_mul(mi[:], mu_sb[:], inv[:])

            # h' = h * inv_replicated (selector matmul for replication)
            hps = []
            for g, rt in enumerate(["RT0", "RT1", "RT2"]):
                rows = 128 if g < 2 else 64
                ps_ir = psr.tile([rows, FD], F32, tag="r")
                nc.tensor.matmul(ps_ir[:], W[rt][:], inv[:])
                hp = sp.tile([rows, FD], BF16, tag=f"hp{g}")
                nc.vector.tensor_mul(hp[:], ps_ir[:], hs[g][:])
                hps.append(hp)

            # ---- fusion MLP (gamma/beta folded; -mu correction via CORR @ mi)
            ps_f1a = psl.tile([128, FD], F32, tag="l")
            ps_f1b = psl.tile([128, FD], F32, tag="l")
            for psf, wl, corr in ((ps_f1a, ["W1A0", "W1A1", "W1A2"], "CORRa"),
                                  (ps_f1b, ["W1B0", "W1B1", "W1B2"], "CORRb")):
                for g in range(3):
                    nc.tensor.matmul(psf[:], W[wl[g]][:], hps[g][:],
                                     start=(g == 0), stop=False)
                nc.tensor.matmul(psf[:], W[corr][:], mi[:],
                                 start=False, stop=True)
            f1a = sp.tile([128, FD], BF16, tag="f1a")
            nc.scalar.activation(f1a[:], ps_f1a[:], AF.Relu, bias=W["b1a"][:])
            f1b = sp.tile([128, FD], BF16, tag="f1b")
            nc.scalar.activation(f1b[:], ps_f1b[:], AF.Relu, bias=W["b1b"][:])

            ps_o1 = psl.tile([128, FD], F32, tag="l")
            nc.tensor.matmul(ps_o1[:], W["W2Aa"][:], f1a[:], start=True, stop=False)
            nc.tensor.matmul(ps_o1[:], W["W2Ba"][:], f1b[:], start=False, stop=True)
            ps_o2 = psl.tile([32, FD], F32, tag="l")
            nc.tensor.matmul(ps_o2[:], W["W2Ab"][:], f1a[:], start=True, stop=False)
            nc.tensor.matmul(ps_o2[:], W["W2Bb"][:], f1b[:], start=False, stop=True)
            oo1 = spo.tile([128, FD], F32, tag="oo1")
            nc.scalar.activation(oo1[:], ps_o1[:], AF.Relu, bias=W["b2a"][:])
            oo2 = spo.tile([32, FD], F32, tag="oo2")
            nc.scalar.activation(oo2[:], ps_o2[:], AF.Relu, bias=W["b2b"][:])

            nc.sync.dma_start(dout[0:128, r0:r0 + FD], oo1[:])
            nc.sync.dma_start(dout[128:160, r0:r0 + FD], oo2[:])

        # software pipeline: emit back(i-1) before front(i) so every
        # engine's in-order queue alternates ready back-work with front
        # work instead of draining a whole iteration at a time
        prev = None
        for it in range(NT + 1):
            if prev is not None:
                emit_back(prev)
            prev = emit_front(it) if it < NT else None

    nc.compile()
    return nc


_CACHE = {}


def _pack_inputs(w):
    """Feature-major bf16 [F_ALL, B] packed input (host-side transpose)."""
    xp = np.zeros((F_ALL, B), dtype=BF16NP)
    for nm, off, f in [("visual", OV, FV), ("audio", OA, FA), ("spatial", OS, FS),
                       ("time", OT, FT), ("pose", OP, FP)]:
        xp[off:off + f, :] = np.asarray(w[nm], np.float32).T.astype(BF16NP)
    return xp


def kernel(**inputs):
    w = {k: np.asarray(v) for k, v in inputs.items()}
    consts = _build_constants(w)
    xpack = _pack_inputs(w)
    blob, const_specs = _pack_blob(consts)

    key = tuple(sorted((k, s, str(d), o, nb)
                       for k, (s, d, o, nb) in const_specs.items()))
    if key not in _CACHE:
        _CACHE[key] = _build_bass(const_specs, blob.shape[1])
    nc = _CACHE[key]

    from concourse.bass_utils import run_bass_kernel_spmd

    in_maps = []
    for c in range(NCORES):
        m = {"xpack": np.ascontiguousarray(xpack[:, c * R:(c + 1) * R]),
             "wblob": blob}
        in_maps.append(m)

    res = run_bass_kernel_spmd(nc, in_maps, core_ids=list(range(NCORES)))
    out = np.concatenate([r["out"] for r in res.results], axis=1)  # [160, B]
    return np.ascontiguousarray(out.T, dtype=np.float32)
